# revision 26
# baseline (speedup 1.0000x reference)
"""Trainium2 Bass kernel for nn_ConstrainedEnhancementModel.

Contract: kernel(**inputs) takes the FULL unsharded inputs (as produced by
reference.setup_inputs()) and returns the FULL [4096, 2000, 6] float32 output.

Strategy (pure data parallel over 8 NeuronCores, 512 batch rows each):
  - Feature-major MLP chain: every hidden activation is stored [feat, batch]
    so torch-layout weights [fan_in, fan_out] are directly the matmul lhsT.
  - x is pre-arranged on the host into the window-blocked layout the kernel
    needs (no on-chip transposes), with the G-bias ones row baked in.
  - W6 (pre-scaled by the per-timestep blend coefficient) is stored fp8-e4m3,
    loaded over the SWDGE ring concurrently with the other loads, and kept
    fully resident in SBUF; the final layer runs DoubleRow fp8 matmuls
    (K=256 per instruction).
  - The constraint/interpolation epilogue is folded into the final matmul:
        out = h5 @ (W6 * c_dec) + x @ G + ones * (b6 * c_dec)
    where G is a sparse constant [600, 12000] matrix holding the linear
    interpolation + anchor/blend coefficients (bf16 path).
  - The output is written in bf16 (upcast to f32 on the host), halving the
    dominant HBM-write traffic; y DMAs alternate between the two HWDGE
    rings (SP / ACT) so neither ring's FIFO drain paces the main loop.
"""

import numpy as np
import ml_dtypes

import concourse.bass as bass
import concourse.bacc as bacc
import concourse.mybir as mybir
import concourse.tile as tile
from concourse import bass_utils

F32 = mybir.dt.float32
BF16 = mybir.dt.bfloat16
FP8 = mybir.dt.float8e4
BF16_NP = ml_dtypes.bfloat16
FP8_NP = ml_dtypes.float8_e4m3

# Problem config (hardcoded; must match the reference)
LOW_T = 100
HIGH_T = 2000
FEAT = 6
HID = 256
NUM_CLASSES = 10
LBL_DIM = 16
UP = 20
B = 4096
NCORES = 8
BC = B // NCORES          # 512 batch rows per core
NBT = BC // 128           # 4 batch tiles per core
D_IN = LOW_T * FEAT       # 600
D_OUT = HIGH_T * FEAT     # 12000
NW = 25                   # output windows (80 timesteps * 6 feats = 480 cols)
WT = 480
NI4 = 7                   # ceil(25/4) groups of 4 windows

DR = mybir.MatmulPerfMode.DoubleRow


def _build_nc():
    """Build the single-core Bass program (SPMD: same program on all 8)."""
    nc = bacc.Bacc("TRN2", target_bir_lowering=False, debug=False)

    xw_d = nc.dram_tensor("xw", [128, NI4 * 512], BF16, kind="ExternalInput")
    xt_d = nc.dram_tensor("xt", [128, 6 * 512], FP8, kind="ExternalInput")
    lab_d = nc.dram_tensor("labf", [1, BC], BF16, kind="ExternalInput")
    w1_d = nc.dram_tensor("w1re", [128, 6 * 512], FP8, kind="ExternalInput")
    w2_d = nc.dram_tensor("w2", [128, 4 * 256], FP8, kind="ExternalInput")
    w3_d = nc.dram_tensor("w3", [128, 2 * 128], FP8, kind="ExternalInput")
    w4_d = nc.dram_tensor("w4", [128, 512], BF16, kind="ExternalInput")
    w5_d = nc.dram_tensor("w5", [128, 2 * 512], FP8, kind="ExternalInput")
    # window-pair-major W6: col block q = 4*window + 2*kp + j holds fp8
    # subtile (2*kp+j) of that window's 480 columns -> DoubleRow pairs sit
    # 480 B apart (small stride keeps the 2-per-cycle rhs fetch alive)
    w6_d = nc.dram_tensor("w6p8", [128, 4 * D_OUT], FP8, kind="ExternalInput")
    bia_d = nc.dram_tensor("bias", [128, 13], F32, kind="ExternalInput")
    emb_d = nc.dram_tensor("embT", [NUM_CLASSES, LBL_DIM], BF16, kind="ExternalInput")
    iota_d = nc.dram_tensor("iota10", [NUM_CLASSES, 1], F32, kind="ExternalInput")
    g_d = nc.dram_tensor("gmat", [128, NI4 * WT], BF16, kind="ExternalInput")
    y_d = nc.dram_tensor("y", [BC, D_OUT], BF16, kind="ExternalOutput")

    RELU = mybir.ActivationFunctionType.Relu

    with tile.TileContext(nc) as tc:
        with (
            tc.tile_pool(name="const", bufs=1) as cp,
            tc.tile_pool(name="outpool", bufs=8) as op,
            tc.tile_pool(name="ppool", bufs=8, space="PSUM") as pm,
        ):
            # ---- persistent SBUF tensors ----
            cw1 = cp.tile([128, 6, 512], FP8, tag="cw1", name="cw1")
            xt = cp.tile([128, 6, 512], FP8, tag="xt", name="xt")
            cw2 = cp.tile([128, 4, 256], FP8, tag="cw2", name="cw2")
            cw3 = cp.tile([128, 2, 128], FP8, tag="cw3", name="cw3")
            cw4 = cp.tile([128, 512], BF16, tag="cw4", name="cw4")
            cw5 = cp.tile([128, 2, 512], FP8, tag="cw5", name="cw5")
            cw6 = cp.tile([128, 4 * NW, WT], FP8, tag="cw6", name="cw6")
            cb = cp.tile([128, 13], F32, tag="cb", name="cb")
            cemb = cp.tile([NUM_CLASSES, LBL_DIM], BF16, tag="cemb", name="cemb")
            ciota = cp.tile([NUM_CLASSES, 1], F32, tag="ciota", name="ciota")
            cg = cp.tile([128, NI4 * WT], BF16, tag="cg", name="cg")
            clab = cp.tile([1, BC], BF16, tag="clab", name="clab")
            ones10 = cp.tile([1, NUM_CLASSES], BF16, tag="ones10", name="ones10")
            xw = cp.tile([128, NI4 * 512], BF16, tag="xw", name="xw")
            h1 = cp.tile([128, 4, BC], FP8, tag="h1", name="h1")
            h2 = cp.tile([128, 2, BC], FP8, tag="h2", name="h2")
            feat = cp.tile([128, BC], BF16, tag="feat", name="feat")
            h4 = cp.tile([128, 2, BC], FP8, tag="h4", name="h4")
            h5 = cp.tile([128, 4, BC], FP8, tag="h5", name="h5")
            onehot = cp.tile([NUM_CLASSES, BC], BF16, tag="onehot", name="onehot")
            embt = cp.tile([LBL_DIM, BC], BF16, tag="embt", name="embt")
            scr = cp.tile([128, 640], BF16, tag="scr", name="scr")

            # bias column layout in cb: b1 m0..3 | b2 m0..1 | b3 | b4 m0..1 | b5 m0..3
            B1, B2, B3, B4, B5 = 0, 4, 6, 7, 9

            # ---- const loads (SP ring; issue order = drain order) ----
            # loads are split across the two HWDGE rings so the two
            # encoder-critical tensors (xt on sync, cw1 on scalar) stream
            # concurrently; xw (only needed by the final phase) and W6 follow
            # on the sync ring.
            nc.sync.dma_start(clab[:], lab_d[:])
            nc.sync.dma_start(xt[:], xt_d[:])
            nc.scalar.dma_start(cw1[:], w1_d[:])
            nc.scalar.dma_start(ciota[:], iota_d[:])
            nc.scalar.dma_start(cemb[:], emb_d[:])
            nc.scalar.dma_start(cw2[:], w2_d[:])
            nc.scalar.dma_start(cw3[:], w3_d[:])
            nc.scalar.dma_start(cw4[:], w4_d[:])
            nc.scalar.dma_start(cw5[:], w5_d[:])
            nc.scalar.dma_start(cb[:], bia_d[:])
            nc.scalar.dma_start(cg[:], g_d[:])
            nc.sync.dma_start(xw[:], xw_d[:])
            # W6 last: the SDMA engines shared-drain everything in flight, so
            # anything issued alongside W6 lands ~6 MB later; the encoder only
            # needs the loads above, and the final layer consumes W6 chunks
            # in issue order anyway.
            nc.gpsimd.memset(scr[:], 0.0)
            nc.gpsimd.memset(ones10[:], 1.0)
            for ks in range(4):
                nc.sync.dma_start(
                    cw6[:, ks * NW:(ks + 1) * NW, :],
                    w6_d[:, ks * D_OUT:(ks + 1) * D_OUT],
                )

            # ---- PE warm-up ----
            # The HAM clock gate holds the PE at 1.2 GHz until it has seen
            # ~3.4 us of sustained FULL-ARRAY activity (skinny matmuls do not
            # register).  These depend only on a gpsimd memset, so they start
            # right after the preamble and heat the PE while xw/cw1 stream
            # in; L1 then runs at the full 2.4 GHz.
            for _ in range(16):
                psw = pm.tile([128, 512], F32, tag="ps", name="ps")
                nc.tensor.matmul(psw[:, :], scr[:, 0:128], scr[:, 128:640],
                                 start=True, stop=True)

            # label one-hot seed: runs while xt/cw1 finish streaming in;
            # the DVE is_equal then overlaps L1
            psl = pm.tile([128, 512], F32, tag="ps", name="ps")
            nc.tensor.matmul(psl[0:NUM_CLASSES, 0:BC], ones10[:], clab[:],
                             start=True, stop=True)
            nc.vector.tensor_scalar(
                onehot[:], psl[0:NUM_CLASSES, 0:BC], ciota[:], None,
                mybir.AluOpType.is_equal,
            )

            # ---- encoder / decoder MLP (feature-major, N = BC) ----
            # L1: [600->512], fp8 DoubleRow over 3 k-subtile pairs
            IDENT = mybir.ActivationFunctionType.Identity
            HB = BC // 2

            def act_split(dst3, msl, ps, bcol, relu=True):
                # layer-boundary activation split across ACT and DVE so the
                # next layer's matmuls are gated by a half-width op
                fn = RELU if relu else IDENT
                nc.scalar.activation(dst3[:, msl, 0:HB], ps[:, 0:HB], fn,
                                     bias=cb[:, bcol:bcol + 1])
                if relu:
                    nc.vector.tensor_scalar(dst3[:, msl, HB:BC], ps[:, HB:BC],
                                            cb[:, bcol:bcol + 1], 0.0,
                                            mybir.AluOpType.add, mybir.AluOpType.max)
                else:
                    nc.vector.tensor_scalar(dst3[:, msl, HB:BC], ps[:, HB:BC],
                                            cb[:, bcol:bcol + 1], None,
                                            mybir.AluOpType.add)

            for m in range(4):
                ps = pm.tile([128, 512], F32, tag="ps", name="ps")
                for kp in range(3):
                    nc.tensor.matmul(
                        ps[:, 0:BC], cw1[:, 2 * kp:2 * kp + 2, m * 128:(m + 1) * 128],
                        xt[:, 2 * kp:2 * kp + 2, :],
                        start=(kp == 0), stop=(kp == 2), perf_mode=DR,
                    )
                act_split(h1, m, ps[:, 0:BC], B1 + m)
            # label embedding: onehot was computed during L1, so this matmul
            # issues with no PE stall
            pse = pm.tile([128, 512], F32, tag="ps", name="ps")
            nc.tensor.matmul(pse[0:LBL_DIM, 0:BC], cemb[:], onehot[:],
                             start=True, stop=True)
            nc.vector.tensor_copy(embt[:], pse[0:LBL_DIM, 0:BC])
            # L2: [512->256], fp8 DR
            for m in range(2):
                ps = pm.tile([128, 512], F32, tag="ps", name="ps")
                for kp in range(2):
                    nc.tensor.matmul(
                        ps[:, 0:BC], cw2[:, 2 * kp:2 * kp + 2, m * 128:(m + 1) * 128],
                        h1[:, 2 * kp:2 * kp + 2, :],
                        start=(kp == 0), stop=(kp == 1), perf_mode=DR,
                    )
                act_split(h2, m, ps[:, 0:BC], B2 + m)
            # L3: [256->128], fp8 DR, no relu
            ps = pm.tile([128, 512], F32, tag="ps", name="ps")
            nc.tensor.matmul(ps[:, 0:BC], cw3[:, 0:2, :], h2[:, 0:2, :],
                             start=True, stop=True, perf_mode=DR)
            nc.scalar.activation(feat[:, 0:HB], ps[:, 0:HB], IDENT, bias=cb[:, B3:B3 + 1])
            nc.vector.tensor_scalar(feat[:, HB:BC], ps[:, HB:BC], cb[:, B3:B3 + 1], None, mybir.AluOpType.add)
            # L4: [144->256] = feat part + label-embedding part (bf16)
            for m in range(2):
                ps = pm.tile([128, 512], F32, tag="ps", name="ps")
                nc.tensor.matmul(ps[:, 0:BC], cw4[:, m * 128:(m + 1) * 128],
                                 feat[:], start=True, stop=False)
                nc.tensor.matmul(ps[:, 0:BC], cw4[0:16, 256 + m * 128:256 + (m + 1) * 128],
                                 embt[:], start=False, stop=True)
                act_split(h4, m, ps[:, 0:BC], B4 + m)
            # L5: [256->512], fp8 DR, output as fp8 k-subtiles of h5
            for m in range(4):
                ps = pm.tile([128, 512], F32, tag="ps", name="ps")
                nc.tensor.matmul(
                    ps[:, 0:BC], cw5[:, 0:2, m * 128:(m + 1) * 128],
                    h4[:, 0:2, :],
                    start=True, stop=True, perf_mode=DR,
                )
                act_split(h5, m, ps[:, 0:BC], B5 + m)

            # ---- final layer + fused constraint epilogue ----
            # W6 fully SBUF-resident (fp8). Per (i4, bt): 4 windows get
            # 2 DoubleRow matmuls each (K=256 per instruction), then the four
            # K=32 G matmuls land on distinct PE row groups (concurrent),
            # then psum -> one [128, 1920] bf16 SBUF tile -> one y DMA,
            # alternating between the SP and ACT HWDGE rings.
            for i4 in range(NI4):
                nwin = 4 if i4 < 6 else 1
                for bt in range(NBT):
                    bsl = slice(bt * 128, (bt + 1) * 128)
                    pss = []
                    for w in range(nwin):
                        pss.append(pm.tile([128, 512], F32, tag="ps", name="ps")[:, 0:WT])
                    for kp in (0, 1):
                        for w in range(nwin):
                            i = 4 * i4 + w
                            nc.tensor.matmul(
                                pss[w][:], h5[:, 2 * kp:2 * kp + 2, bsl],
                                cw6[:, 4 * i + 2 * kp:4 * i + 2 * kp + 2, :],
                                start=(kp == 0), stop=False, perf_mode=DR,
                            )
                    for w in range(nwin):
                        p0 = 32 * w
                        nc.tensor.matmul(
                            pss[w][:],
                            xw[p0:p0 + 32, i4 * 512 + bt * 128:i4 * 512 + (bt + 1) * 128],
                            cg[p0:p0 + 32, i4 * WT:(i4 + 1) * WT],
                            start=False, stop=True, tile_position=(p0, 0),
                        )
                    ob = op.tile([128, 4 * WT], BF16, tag="ob", name="ob")
                    for w in range(nwin):
                        if w % 2 == 0:
                            nc.vector.tensor_copy(ob[:, w * WT:(w + 1) * WT], pss[w][:])
                        else:
                            nc.scalar.copy(ob[:, w * WT:(w + 1) * WT], pss[w][:])
                    eng = nc.scalar if (i4 == NI4 - 1 and bt % 2 == 1) else nc.sync
                    eng.dma_start(
                        y_d[bsl, i4 * 4 * WT:i4 * 4 * WT + nwin * WT],
                        ob[:, 0:nwin * WT],
                    )

    nc.compile()
    return nc


def _host_prep(inputs):
    """Build per-core in_maps from the full inputs."""
    x_full = np.asarray(inputs["low_res_data"], np.float32).reshape(B, D_IN)
    labels = np.asarray(inputs["labels"]).astype(np.float32)
    W1 = np.asarray(inputs["W1"], np.float32)
    W6 = np.asarray(inputs["W6"], np.float32)
    b6 = np.asarray(inputs["b6"], np.float32)

    # per-timestep blend coefficients (match the reference formulas)
    t = np.arange(HIGH_T)
    seg = np.clip(t // UP, 0, LOW_T - 2)
    alpha = ((t - seg * UP) / UP).astype(np.float64)
    is_anchor = (t % UP) == 0
    interior = t < (LOW_T - 1) * UP
    blendf = np.where(is_anchor, 1.0, np.where(interior, 0.8, 0.0))
    c_d = np.where(is_anchor, 0.0, np.where(interior, 0.2, 1.0))
    c_start = blendf * (1.0 - alpha)
    c_end = blendf * alpha

    # G matrix, window-blocked: [128, NI4*480]; window i lives at partition
    # offset 32*(i%4), col block i//4.  Rows r=0..29 <-> x col 24*i + r,
    # row 30 = bias row (paired with the constant-1.0 row of xw).
    gmat = np.zeros((128, NI4 * WT), np.float64)
    for tt in range(HIGH_T):
        i, dt = divmod(tt, 80)
        i4, wpos = divmod(i, 4)
        p0 = 32 * wpos
        sl = seg[tt] - 4 * i
        for f in range(FEAT):
            col = i4 * WT + FEAT * dt + f
            gmat[p0 + FEAT * sl + f, col] += c_start[tt]
            gmat[p0 + FEAT * (sl + 1) + f, col] += c_end[tt]
            gmat[p0 + 30, col] = c_d[tt] * np.float64(b6[FEAT * tt + f])
    gmat = gmat.astype(np.float32).astype(BF16_NP)

    c_d_full = np.repeat(c_d, FEAT).astype(np.float32)
    # window-pair-major fp8 W6: [s=subtile, p, i=window, c] -> [p, i, s, c]
    w6p = (
        (W6 * c_d_full[None, :]).astype(FP8_NP)
        .reshape(4, 128, NW, WT).transpose(1, 2, 0, 3).reshape(128, 4 * D_OUT)
        .copy()
    )

    # W1 padded to 768 contraction rows (6 k-subtiles = 3 DoubleRow pairs)
    w1p = np.zeros((768, 512), np.float32)
    w1p[:D_IN] = W1
    w1re = w1p.reshape(6, 128, 512).transpose(1, 0, 2).reshape(128, 6 * 512).copy().astype(FP8_NP)

    w4 = np.zeros((128, 512), np.float32)
    w4[:, 0:256] = np.asarray(inputs["W4"], np.float32)[:128]
    w4[0:16, 256:512] = np.asarray(inputs["W4"], np.float32)[128:144]

    bias = np.zeros((128, 13), np.float32)
    bias[:, 0:4] = np.asarray(inputs["b1"], np.float32).reshape(4, 128).T
    bias[:, 4:6] = np.asarray(inputs["b2"], np.float32).reshape(2, 128).T
    bias[:, 6] = np.asarray(inputs["b3"], np.float32)
    bias[:, 7:9] = np.asarray(inputs["b4"], np.float32).reshape(2, 128).T
    bias[:, 9:13] = np.asarray(inputs["b5"], np.float32).reshape(4, 128).T

    const_map = {
        "w1re": w1re,
        "w2": np.asarray(inputs["W2"], np.float32).reshape(4, 128, 256).transpose(1, 0, 2).reshape(128, 1024).copy().astype(FP8_NP),
        "w3": np.asarray(inputs["W3"], np.float32).reshape(2, 128, 128).transpose(1, 0, 2).reshape(128, 256).copy().astype(FP8_NP),
        "w4": w4.astype(BF16_NP),
        "w5": np.asarray(inputs["W5"], np.float32).reshape(2, 128, 512).transpose(1, 0, 2).reshape(128, 1024).copy().astype(FP8_NP),
        "w6p8": w6p,
        "bias": bias,
        "embT": np.asarray(inputs["emb"], np.float32).astype(BF16_NP),
        "iota10": np.arange(NUM_CLASSES, dtype=np.float32).reshape(NUM_CLASSES, 1),
        "gmat": gmat,
    }

    # window-blocked x layout: [128, NI4*512]; window i = 4*i4 + wpos:
    # partition 32*wpos + r (r<30) = x col 24*i + r; row 30 = 1.0 (G bias);
    # row 31 = 0.  Column = i4*512 + batch row within the core chunk.
    in_maps = []
    for c in range(NCORES):
        sl = slice(c * BC, (c + 1) * BC)
        xc = x_full[sl]                                    # [BC, 600]
        xwin = np.zeros((128, NI4 * 512), np.float32)
        for i4 in range(NI4):
            nwin = 4 if i4 < 6 else 1
            blk = xwin[:, i4 * 512:(i4 + 1) * 512]
            for wpos in range(nwin):
                i = 4 * i4 + wpos
                c0 = 24 * i
                ncols = min(30, D_IN - c0)
                blk[32 * wpos:32 * wpos + ncols, :] = xc[:, c0:c0 + ncols].T
                blk[32 * wpos + 30, :] = 1.0
        xp = np.zeros((768, BC), np.float32)
        xp[:D_IN] = xc.T
        m = dict(const_map)
        m["xt"] = xp.reshape(6, 128, BC).transpose(1, 0, 2).reshape(128, 6 * BC).copy().astype(FP8_NP)
        m["xw"] = xwin.astype(BF16_NP)
        m["labf"] = labels[sl].reshape(1, BC).astype(BF16_NP)
        in_maps.append(m)
    return in_maps


_NC_CACHE = None


def kernel(**inputs) -> np.ndarray:
    global _NC_CACHE
    if _NC_CACHE is None:
        _NC_CACHE = _build_nc()
    nc = _NC_CACHE
    in_maps = _host_prep(inputs)
    res = bass_utils.run_bass_kernel_spmd(nc, in_maps, core_ids=list(range(NCORES)))
    out = np.concatenate(
        [np.asarray(res.results[c]["y"]).astype(np.float32) for c in range(NCORES)],
        axis=0,
    )
    return out.reshape(B, HIGH_T, FEAT)


# revision 27
# speedup vs baseline: 1.0286x; 1.0286x over previous
"""Trainium2 Bass kernel for nn_ConstrainedEnhancementModel.

Contract: kernel(**inputs) takes the FULL unsharded inputs (as produced by
reference.setup_inputs()) and returns the FULL [4096, 2000, 6] float32 output.

Strategy (pure data parallel over 8 NeuronCores, 512 batch rows each):
  - Feature-major MLP chain: every hidden activation is stored [feat, batch]
    so torch-layout weights [fan_in, fan_out] are directly the matmul lhsT.
  - x is pre-arranged on the host into the window-blocked layout the kernel
    needs (no on-chip transposes), with the G-bias ones row baked in.
  - W6 (pre-scaled by the per-timestep blend coefficient) is stored fp8-e4m3,
    loaded over the SWDGE ring concurrently with the other loads, and kept
    fully resident in SBUF; the final layer runs DoubleRow fp8 matmuls
    (K=256 per instruction).
  - The constraint/interpolation epilogue is folded into the final matmul:
        out = h5 @ (W6 * c_dec) + x @ G + ones * (b6 * c_dec)
    where G is a sparse constant [600, 12000] matrix holding the linear
    interpolation + anchor/blend coefficients (bf16 path).
  - The output is written in bf16 (upcast to f32 on the host), halving the
    dominant HBM-write traffic; y DMAs alternate between the two HWDGE
    rings (SP / ACT) so neither ring's FIFO drain paces the main loop.
"""

import numpy as np
import ml_dtypes

import concourse.bass as bass
import concourse.bacc as bacc
import concourse.mybir as mybir
import concourse.tile as tile
from concourse import bass_utils

F32 = mybir.dt.float32
BF16 = mybir.dt.bfloat16
FP8 = mybir.dt.float8e4
BF16_NP = ml_dtypes.bfloat16
FP8_NP = ml_dtypes.float8_e4m3

# Problem config (hardcoded; must match the reference)
LOW_T = 100
HIGH_T = 2000
FEAT = 6
HID = 256
NUM_CLASSES = 10
LBL_DIM = 16
UP = 20
B = 4096
NCORES = 8
BC = B // NCORES          # 512 batch rows per core
NBT = BC // 128           # 4 batch tiles per core
D_IN = LOW_T * FEAT       # 600
D_OUT = HIGH_T * FEAT     # 12000
NW = 25                   # output windows (80 timesteps * 6 feats = 480 cols)
WT = 480
NI4 = 7                   # ceil(25/4) groups of 4 windows

DR = mybir.MatmulPerfMode.DoubleRow


def _build_nc():
    """Build the single-core Bass program (SPMD: same program on all 8)."""
    nc = bacc.Bacc("TRN2", target_bir_lowering=False, debug=False)

    xw_d = nc.dram_tensor("xw", [128, NI4 * 512], BF16, kind="ExternalInput")
    xt_d = nc.dram_tensor("xt", [128, 6 * 512], FP8, kind="ExternalInput")
    lab_d = nc.dram_tensor("labf", [1, BC], BF16, kind="ExternalInput")
    w1_d = nc.dram_tensor("w1re", [128, 6 * 512], FP8, kind="ExternalInput")
    w2_d = nc.dram_tensor("w2", [128, 4 * 256], FP8, kind="ExternalInput")
    w3_d = nc.dram_tensor("w3", [128, 2 * 128], FP8, kind="ExternalInput")
    w4_d = nc.dram_tensor("w4", [128, 512], BF16, kind="ExternalInput")
    w5_d = nc.dram_tensor("w5", [128, 2 * 512], FP8, kind="ExternalInput")
    # window-pair-major W6: col block q = 4*window + 2*kp + j holds fp8
    # subtile (2*kp+j) of that window's 480 columns -> DoubleRow pairs sit
    # 480 B apart (small stride keeps the 2-per-cycle rhs fetch alive)
    w6_d = nc.dram_tensor("w6p8", [128, 4 * D_OUT], FP8, kind="ExternalInput")
    bia_d = nc.dram_tensor("bias", [128, 13], F32, kind="ExternalInput")
    emb_d = nc.dram_tensor("embT", [NUM_CLASSES, LBL_DIM], BF16, kind="ExternalInput")
    iota_d = nc.dram_tensor("iota10", [NUM_CLASSES, 1], F32, kind="ExternalInput")
    g_d = nc.dram_tensor("gmat", [128, NI4 * WT], BF16, kind="ExternalInput")
    y_d = nc.dram_tensor("y", [BC, D_OUT], BF16, kind="ExternalOutput")

    RELU = mybir.ActivationFunctionType.Relu

    with tile.TileContext(nc) as tc:
        with (
            tc.tile_pool(name="const", bufs=1) as cp,
            tc.tile_pool(name="outpool", bufs=8) as op,
            tc.tile_pool(name="ppool", bufs=8, space="PSUM") as pm,
        ):
            # ---- persistent SBUF tensors ----
            cw1 = cp.tile([128, 6, 512], FP8, tag="cw1", name="cw1")
            xt = cp.tile([128, 6, 512], FP8, tag="xt", name="xt")
            cw2 = cp.tile([128, 4, 256], FP8, tag="cw2", name="cw2")
            cw3 = cp.tile([128, 2, 128], FP8, tag="cw3", name="cw3")
            cw4 = cp.tile([128, 512], BF16, tag="cw4", name="cw4")
            cw5 = cp.tile([128, 2, 512], FP8, tag="cw5", name="cw5")
            cw6 = cp.tile([128, 4 * NW, WT], FP8, tag="cw6", name="cw6")
            cb = cp.tile([128, 13], F32, tag="cb", name="cb")
            cemb = cp.tile([NUM_CLASSES, LBL_DIM], BF16, tag="cemb", name="cemb")
            ciota = cp.tile([NUM_CLASSES, 1], F32, tag="ciota", name="ciota")
            cg = cp.tile([128, NI4 * WT], BF16, tag="cg", name="cg")
            clab = cp.tile([1, BC], BF16, tag="clab", name="clab")
            ones10 = cp.tile([1, NUM_CLASSES], BF16, tag="ones10", name="ones10")
            xw = cp.tile([128, NI4 * 512], BF16, tag="xw", name="xw")
            h1 = cp.tile([128, 4, BC], FP8, tag="h1", name="h1")
            h2 = cp.tile([128, 2, BC], FP8, tag="h2", name="h2")
            feat = cp.tile([128, BC], BF16, tag="feat", name="feat")
            h4 = cp.tile([128, 2, BC], FP8, tag="h4", name="h4")
            h5 = cp.tile([128, 4, BC], FP8, tag="h5", name="h5")
            onehot = cp.tile([NUM_CLASSES, BC], BF16, tag="onehot", name="onehot")
            embt = cp.tile([LBL_DIM, BC], BF16, tag="embt", name="embt")
            scr = cp.tile([128, 640], BF16, tag="scr", name="scr")

            # bias column layout in cb: b1 m0..3 | b2 m0..1 | b3 | b4 m0..1 | b5 m0..3
            B1, B2, B3, B4, B5 = 0, 4, 6, 7, 9

            # ---- const loads (SP ring; issue order = drain order) ----
            # loads are split across the two HWDGE rings so the two
            # encoder-critical tensors (xt on sync, cw1 on scalar) stream
            # concurrently; xw (only needed by the final phase) and W6 follow
            # on the sync ring.
            nc.sync.dma_start(clab[:], lab_d[:])
            nc.sync.dma_start(cb[:], bia_d[:])
            nc.sync.dma_start(xt[:], xt_d[:])
            nc.scalar.dma_start(ciota[:], iota_d[:])
            nc.scalar.dma_start(cemb[:], emb_d[:])
            nc.scalar.dma_start(cw1[:], w1_d[:])
            nc.scalar.dma_start(cw2[:], w2_d[:])
            nc.scalar.dma_start(cw3[:], w3_d[:])
            nc.scalar.dma_start(cw4[:], w4_d[:])
            nc.scalar.dma_start(cw5[:], w5_d[:])
            nc.scalar.dma_start(cg[:], g_d[:])
            nc.sync.dma_start(xw[:], xw_d[:])
            # W6 last: the SDMA engines shared-drain everything in flight, so
            # anything issued alongside W6 lands ~6 MB later; the encoder only
            # needs the loads above, and the final layer consumes W6 chunks
            # in issue order anyway.
            nc.gpsimd.memset(scr[:], 0.0)
            nc.gpsimd.memset(ones10[:], 1.0)
            for ks in range(4):
                nc.sync.dma_start(
                    cw6[:, ks * NW:(ks + 1) * NW, :],
                    w6_d[:, ks * D_OUT:(ks + 1) * D_OUT],
                )

            # ---- PE warm-up ----
            # The HAM clock gate holds the PE at 1.2 GHz until it has seen
            # ~3.4 us of sustained FULL-ARRAY activity (skinny matmuls do not
            # register).  These depend only on a gpsimd memset, so they start
            # right after the preamble and heat the PE while xw/cw1 stream
            # in; L1 then runs at the full 2.4 GHz.
            for _ in range(16):
                psw = pm.tile([128, 512], F32, tag="ps", name="ps")
                nc.tensor.matmul(psw[:, :], scr[:, 0:128], scr[:, 128:640],
                                 start=True, stop=True)

            # label one-hot seed: runs while xt/cw1 finish streaming in;
            # the DVE is_equal then overlaps L1
            psl = pm.tile([128, 512], F32, tag="ps", name="ps")
            nc.tensor.matmul(psl[0:NUM_CLASSES, 0:BC], ones10[:], clab[:],
                             start=True, stop=True)
            nc.vector.tensor_scalar(
                onehot[:], psl[0:NUM_CLASSES, 0:BC], ciota[:], None,
                mybir.AluOpType.is_equal,
            )

            # ---- encoder / decoder MLP (feature-major, N = BC) ----
            # L1: [600->512], fp8 DoubleRow over 3 k-subtile pairs
            IDENT = mybir.ActivationFunctionType.Identity
            HB = BC // 2

            def act_split(dst3, msl, ps, bcol, relu=True):
                # layer-boundary activation split across ACT and DVE so the
                # next layer's matmuls are gated by a half-width op
                fn = RELU if relu else IDENT
                nc.scalar.activation(dst3[:, msl, 0:HB], ps[:, 0:HB], fn,
                                     bias=cb[:, bcol:bcol + 1])
                if relu:
                    nc.vector.tensor_scalar(dst3[:, msl, HB:BC], ps[:, HB:BC],
                                            cb[:, bcol:bcol + 1], 0.0,
                                            mybir.AluOpType.add, mybir.AluOpType.max)
                else:
                    nc.vector.tensor_scalar(dst3[:, msl, HB:BC], ps[:, HB:BC],
                                            cb[:, bcol:bcol + 1], None,
                                            mybir.AluOpType.add)

            for m in range(4):
                ps = pm.tile([128, 512], F32, tag="ps", name="ps")
                for kp in range(3):
                    nc.tensor.matmul(
                        ps[:, 0:BC], cw1[:, 2 * kp:2 * kp + 2, m * 128:(m + 1) * 128],
                        xt[:, 2 * kp:2 * kp + 2, :],
                        start=(kp == 0), stop=(kp == 2), perf_mode=DR,
                    )
                act_split(h1, m, ps[:, 0:BC], B1 + m)
            # label embedding: onehot was computed during L1, so this matmul
            # issues with no PE stall
            pse = pm.tile([128, 512], F32, tag="ps", name="ps")
            nc.tensor.matmul(pse[0:LBL_DIM, 0:BC], cemb[:], onehot[:],
                             start=True, stop=True)
            nc.vector.tensor_copy(embt[:], pse[0:LBL_DIM, 0:BC])
            # L2: [512->256], fp8 DR
            for m in range(2):
                ps = pm.tile([128, 512], F32, tag="ps", name="ps")
                for kp in range(2):
                    nc.tensor.matmul(
                        ps[:, 0:BC], cw2[:, 2 * kp:2 * kp + 2, m * 128:(m + 1) * 128],
                        h1[:, 2 * kp:2 * kp + 2, :],
                        start=(kp == 0), stop=(kp == 1), perf_mode=DR,
                    )
                act_split(h2, m, ps[:, 0:BC], B2 + m)
            # L3: [256->128], fp8 DR, no relu
            ps = pm.tile([128, 512], F32, tag="ps", name="ps")
            nc.tensor.matmul(ps[:, 0:BC], cw3[:, 0:2, :], h2[:, 0:2, :],
                             start=True, stop=True, perf_mode=DR)
            nc.scalar.activation(feat[:, 0:HB], ps[:, 0:HB], IDENT, bias=cb[:, B3:B3 + 1])
            nc.vector.tensor_scalar(feat[:, HB:BC], ps[:, HB:BC], cb[:, B3:B3 + 1], None, mybir.AluOpType.add)
            # L4: [144->256] = feat part + label-embedding part (bf16)
            for m in range(2):
                ps = pm.tile([128, 512], F32, tag="ps", name="ps")
                nc.tensor.matmul(ps[:, 0:BC], cw4[:, m * 128:(m + 1) * 128],
                                 feat[:], start=True, stop=False)
                nc.tensor.matmul(ps[:, 0:BC], cw4[0:16, 256 + m * 128:256 + (m + 1) * 128],
                                 embt[:], start=False, stop=True)
                act_split(h4, m, ps[:, 0:BC], B4 + m)
            # L5: [256->512], fp8 DR, output as fp8 k-subtiles of h5
            for m in range(4):
                ps = pm.tile([128, 512], F32, tag="ps", name="ps")
                nc.tensor.matmul(
                    ps[:, 0:BC], cw5[:, 0:2, m * 128:(m + 1) * 128],
                    h4[:, 0:2, :],
                    start=True, stop=True, perf_mode=DR,
                )
                act_split(h5, m, ps[:, 0:BC], B5 + m)

            # ---- final layer + fused constraint epilogue ----
            # W6 fully SBUF-resident (fp8). Per (i4, bt): 4 windows get
            # 2 DoubleRow matmuls each (K=256 per instruction), then the four
            # K=32 G matmuls land on distinct PE row groups (concurrent),
            # then psum -> one [128, 1920] bf16 SBUF tile -> one y DMA,
            # alternating between the SP and ACT HWDGE rings.
            for i4 in range(NI4):
                nwin = 4 if i4 < 6 else 1
                for bt in range(NBT):
                    bsl = slice(bt * 128, (bt + 1) * 128)
                    pss = []
                    for w in range(nwin):
                        pss.append(pm.tile([128, 512], F32, tag="ps", name="ps")[:, 0:WT])
                    for kp in (0, 1):
                        for w in range(nwin):
                            i = 4 * i4 + w
                            nc.tensor.matmul(
                                pss[w][:], h5[:, 2 * kp:2 * kp + 2, bsl],
                                cw6[:, 4 * i + 2 * kp:4 * i + 2 * kp + 2, :],
                                start=(kp == 0), stop=False, perf_mode=DR,
                            )
                    for w in range(nwin):
                        p0 = 32 * w
                        nc.tensor.matmul(
                            pss[w][:],
                            xw[p0:p0 + 32, i4 * 512 + bt * 128:i4 * 512 + (bt + 1) * 128],
                            cg[p0:p0 + 32, i4 * WT:(i4 + 1) * WT],
                            start=False, stop=True, tile_position=(p0, 0),
                        )
                    ob = op.tile([128, 4 * WT], BF16, tag="ob", name="ob")
                    for w in range(nwin):
                        if w % 2 == 0:
                            nc.vector.tensor_copy(ob[:, w * WT:(w + 1) * WT], pss[w][:])
                        else:
                            nc.scalar.copy(ob[:, w * WT:(w + 1) * WT], pss[w][:])
                    eng = nc.scalar if (i4 == NI4 - 1 and bt % 2 == 1) else nc.sync
                    eng.dma_start(
                        y_d[bsl, i4 * 4 * WT:i4 * 4 * WT + nwin * WT],
                        ob[:, 0:nwin * WT],
                    )

    nc.compile()
    return nc


def _host_prep(inputs):
    """Build per-core in_maps from the full inputs."""
    x_full = np.asarray(inputs["low_res_data"], np.float32).reshape(B, D_IN)
    labels = np.asarray(inputs["labels"]).astype(np.float32)
    W1 = np.asarray(inputs["W1"], np.float32)
    W6 = np.asarray(inputs["W6"], np.float32)
    b6 = np.asarray(inputs["b6"], np.float32)

    # per-timestep blend coefficients (match the reference formulas)
    t = np.arange(HIGH_T)
    seg = np.clip(t // UP, 0, LOW_T - 2)
    alpha = ((t - seg * UP) / UP).astype(np.float64)
    is_anchor = (t % UP) == 0
    interior = t < (LOW_T - 1) * UP
    blendf = np.where(is_anchor, 1.0, np.where(interior, 0.8, 0.0))
    c_d = np.where(is_anchor, 0.0, np.where(interior, 0.2, 1.0))
    c_start = blendf * (1.0 - alpha)
    c_end = blendf * alpha

    # G matrix, window-blocked: [128, NI4*480]; window i lives at partition
    # offset 32*(i%4), col block i//4.  Rows r=0..29 <-> x col 24*i + r,
    # row 30 = bias row (paired with the constant-1.0 row of xw).
    gmat = np.zeros((128, NI4 * WT), np.float64)
    for tt in range(HIGH_T):
        i, dt = divmod(tt, 80)
        i4, wpos = divmod(i, 4)
        p0 = 32 * wpos
        sl = seg[tt] - 4 * i
        for f in range(FEAT):
            col = i4 * WT + FEAT * dt + f
            gmat[p0 + FEAT * sl + f, col] += c_start[tt]
            gmat[p0 + FEAT * (sl + 1) + f, col] += c_end[tt]
            gmat[p0 + 30, col] = c_d[tt] * np.float64(b6[FEAT * tt + f])
    gmat = gmat.astype(np.float32).astype(BF16_NP)

    c_d_full = np.repeat(c_d, FEAT).astype(np.float32)
    # window-pair-major fp8 W6: [s=subtile, p, i=window, c] -> [p, i, s, c]
    w6p = (
        (W6 * c_d_full[None, :]).astype(FP8_NP)
        .reshape(4, 128, NW, WT).transpose(1, 2, 0, 3).reshape(128, 4 * D_OUT)
        .copy()
    )

    # W1 padded to 768 contraction rows (6 k-subtiles = 3 DoubleRow pairs)
    w1p = np.zeros((768, 512), np.float32)
    w1p[:D_IN] = W1
    w1re = w1p.reshape(6, 128, 512).transpose(1, 0, 2).reshape(128, 6 * 512).copy().astype(FP8_NP)

    w4 = np.zeros((128, 512), np.float32)
    w4[:, 0:256] = np.asarray(inputs["W4"], np.float32)[:128]
    w4[0:16, 256:512] = np.asarray(inputs["W4"], np.float32)[128:144]

    bias = np.zeros((128, 13), np.float32)
    bias[:, 0:4] = np.asarray(inputs["b1"], np.float32).reshape(4, 128).T
    bias[:, 4:6] = np.asarray(inputs["b2"], np.float32).reshape(2, 128).T
    bias[:, 6] = np.asarray(inputs["b3"], np.float32)
    bias[:, 7:9] = np.asarray(inputs["b4"], np.float32).reshape(2, 128).T
    bias[:, 9:13] = np.asarray(inputs["b5"], np.float32).reshape(4, 128).T

    const_map = {
        "w1re": w1re,
        "w2": np.asarray(inputs["W2"], np.float32).reshape(4, 128, 256).transpose(1, 0, 2).reshape(128, 1024).copy().astype(FP8_NP),
        "w3": np.asarray(inputs["W3"], np.float32).reshape(2, 128, 128).transpose(1, 0, 2).reshape(128, 256).copy().astype(FP8_NP),
        "w4": w4.astype(BF16_NP),
        "w5": np.asarray(inputs["W5"], np.float32).reshape(2, 128, 512).transpose(1, 0, 2).reshape(128, 1024).copy().astype(FP8_NP),
        "w6p8": w6p,
        "bias": bias,
        "embT": np.asarray(inputs["emb"], np.float32).astype(BF16_NP),
        "iota10": np.arange(NUM_CLASSES, dtype=np.float32).reshape(NUM_CLASSES, 1),
        "gmat": gmat,
    }

    # window-blocked x layout: [128, NI4*512]; window i = 4*i4 + wpos:
    # partition 32*wpos + r (r<30) = x col 24*i + r; row 30 = 1.0 (G bias);
    # row 31 = 0.  Column = i4*512 + batch row within the core chunk.
    in_maps = []
    for c in range(NCORES):
        sl = slice(c * BC, (c + 1) * BC)
        xc = x_full[sl]                                    # [BC, 600]
        xwin = np.zeros((128, NI4 * 512), np.float32)
        for i4 in range(NI4):
            nwin = 4 if i4 < 6 else 1
            blk = xwin[:, i4 * 512:(i4 + 1) * 512]
            for wpos in range(nwin):
                i = 4 * i4 + wpos
                c0 = 24 * i
                ncols = min(30, D_IN - c0)
                blk[32 * wpos:32 * wpos + ncols, :] = xc[:, c0:c0 + ncols].T
                blk[32 * wpos + 30, :] = 1.0
        xp = np.zeros((768, BC), np.float32)
        xp[:D_IN] = xc.T
        m = dict(const_map)
        m["xt"] = xp.reshape(6, 128, BC).transpose(1, 0, 2).reshape(128, 6 * BC).copy().astype(FP8_NP)
        m["xw"] = xwin.astype(BF16_NP)
        m["labf"] = labels[sl].reshape(1, BC).astype(BF16_NP)
        in_maps.append(m)
    return in_maps


_NC_CACHE = None


def kernel(**inputs) -> np.ndarray:
    global _NC_CACHE
    if _NC_CACHE is None:
        _NC_CACHE = _build_nc()
    nc = _NC_CACHE
    in_maps = _host_prep(inputs)
    res = bass_utils.run_bass_kernel_spmd(nc, in_maps, core_ids=list(range(NCORES)))
    out = np.concatenate(
        [np.asarray(res.results[c]["y"]).astype(np.float32) for c in range(NCORES)],
        axis=0,
    )
    return out.reshape(B, HIGH_T, FEAT)
